# revision 3
# baseline (speedup 1.0000x reference)
"""GRU-style GNN message-passing kernel for Trainium2 (8 NeuronCores, SPMD).

Reference computation (per node b, features 256, 8 neighbors):
    xr = x @ Wir.T + bir
    hr_n = hs_n @ Whr.T + bhr
    r_n = sigmoid(xr + hr_n)
    z = sigmoid(x @ Wiz.T + biz + h_sum @ Whz.T + bhz)
    s = sum_n r_n * hs_n
    n = tanh(x @ Win.T + bin + s @ Whn.T + bhn)
    out = (1 - z) * n + z * h_sum

Strategy: data-parallel over the node dim B=32768 across 8 cores (4096
rows each), batch-chunked 8x512 per core. Everything on-chip runs in
feature-major ("transposed") layout [256 features (2 partition chunks
of 128), batch free dim], so every linear layer is a natural PE matmul
with float32r operands (full PE rate at N=512, ~11-bit mantissa,
fp32 PSUM accumulation). Engine placement per chunk:
  - PE: all 13 linear-layer matmuls; the shared xr term is added into
    each neighbor's PSUM group via an identity matmul; the z and n
    gates accumulate both their linear terms directly in PSUM.
  - ACT: sigmoid/tanh with the combined bias folded in (bias is
    per-partition = per-feature in this layout); PSUM->SBUF copies.
  - DVE: one big in-place multiply r_cat *= hs_cat [128, 8192] and one
    tensor_reduce over the neighbor axis -> s [128, 1024].
  - GPSIMD: the final gate combine out = n + z*(h - n) on [128, 1024].
"""

import sys
import numpy as np
from contextlib import ExitStack

sys.path.insert(0, "/opt/trn_rl_repo")

import concourse.bacc as bacc
import concourse.tile as tile
from concourse import mybir
from concourse.bass_utils import run_bass_kernel_spmd

F32 = mybir.dt.float32
F32R = mybir.dt.float32r

N_NEIGH, B, IN, H = 8, 32768, 256, 256
M = 8                    # cores
BL = B // M              # rows per core (4096)
NCH = 8                  # batch chunks per core
CW = BL // NCH           # chunk width (512)

_cached = None  # compiled program, reused across kernel() calls


def _build():
    nc = bacc.Bacc("TRN2", target_bir_lowering=False, debug=False, num_devices=M)

    xT = nc.dram_tensor("xT", [IN, BL], F32R, kind="ExternalInput").ap()
    hT = nc.dram_tensor("hT", [H, BL], F32R, kind="ExternalInput").ap()
    hsT = nc.dram_tensor("hsT", [N_NEIGH, H, BL], F32R, kind="ExternalInput").ap()
    wAP = {}
    for w in ("wir", "whr", "wiz", "whz", "win", "whn"):
        wAP[w] = nc.dram_tensor(w, [256, 256], F32R, kind="ExternalInput").ap()
    ident = nc.dram_tensor("ident", [128, 128], F32R, kind="ExternalInput").ap()
    # bias pack: col f*3+j holds feature-chunk f of (b_r, b_z, b_n)[j]
    biasp = nc.dram_tensor("biasp", [128, 6], F32, kind="ExternalInput").ap()
    outT = nc.dram_tensor("outT", [H, BL], F32, kind="ExternalOutput").ap()

    with tile.TileContext(nc) as tc, ExitStack() as ctx:
        const_pool = ctx.enter_context(tc.tile_pool(name="const", bufs=1))
        x_pool = ctx.enter_context(tc.tile_pool(name="x", bufs=2))
        h_pool = ctx.enter_context(tc.tile_pool(name="h", bufs=2))
        hs_pool = ctx.enter_context(tc.tile_pool(name="hs", bufs=2))
        xr_pool = ctx.enter_context(tc.tile_pool(name="xr", bufs=2))
        z_pool = ctx.enter_context(tc.tile_pool(name="z", bufs=2))
        s_pool = ctx.enter_context(tc.tile_pool(name="s", bufs=2))
        r_pool = ctx.enter_context(tc.tile_pool(name="r", bufs=1))
        n_pool = ctx.enter_context(tc.tile_pool(name="n", bufs=2))
        d_pool = ctx.enter_context(tc.tile_pool(name="d", bufs=2))
        o_pool = ctx.enter_context(tc.tile_pool(name="o", bufs=2))
        pz_pool = ctx.enter_context(tc.tile_pool(name="pz", bufs=2, space="PSUM"))
        pr_pool = ctx.enter_context(tc.tile_pool(name="pr", bufs=4, space="PSUM"))
        pn_pool = ctx.enter_context(tc.tile_pool(name="pn", bufs=2, space="PSUM"))

        # --- constants ---
        wt = {}
        for w in ("wir", "whr", "wiz", "whz", "win", "whn"):
            wt[w] = []
            for k in range(2):
                t = const_pool.tile([128, 256], F32R, tag=f"{w}{k}", name=f"{w}{k}")
                nc.sync.dma_start(out=t[:, :], in_=wAP[w][k * 128:(k + 1) * 128, :])
                wt[w].append(t)
        id_t = const_pool.tile([128, 128], F32R, tag="ident", name="id_t")
        nc.sync.dma_start(out=id_t[:, :], in_=ident[:, :])
        bias_t = const_pool.tile([128, 6], F32, tag="biasp", name="bias_t")
        nc.sync.dma_start(out=bias_t[:, :], in_=biasp[:, :])

        def fcols(t, f):
            return t[:, f * 128:(f + 1) * 128]

        for c in range(NCH):
            sl = slice(c * CW, (c + 1) * CW)

            # x.T chunks: [128, 512] per feature chunk
            xt = []
            for k in range(2):
                t = x_pool.tile([128, CW], F32R, tag=f"x{k}", name=f"x{k}_{c}")
                nc.sync.dma_start(out=t[:, :], in_=xT[k * 128:(k + 1) * 128, sl])
                xt.append(t)
            # h_sum.T as one [128, 1024] tile, f-chunk halves
            ht = h_pool.tile([128, 2 * CW], F32R, tag="h", name=f"h_{c}")
            for k in range(2):
                nc.sync.dma_start(out=ht[:, k * CW:(k + 1) * CW],
                                  in_=hT[k * 128:(k + 1) * 128, sl])
            # hs.T as one [128, 8192] tile: layout (f, n, b)
            hsc = hs_pool.tile([128, 2 * N_NEIGH * CW], F32R, tag="hs",
                               name=f"hs_{c}")
            for f in range(2):
                for n in range(N_NEIGH):
                    nc.sync.dma_start(
                        out=hsc[:, (f * N_NEIGH + n) * CW:(f * N_NEIGH + n + 1) * CW],
                        in_=hsT[n, f * 128:(f + 1) * 128, sl])

            def hs_slice(f, n):
                return hsc[:, (f * N_NEIGH + n) * CW:(f * N_NEIGH + n + 1) * CW]

            # --- z gate: sigmoid(Wiz@x + Whz@h + b_z), PSUM-accumulated ---
            zt = z_pool.tile([128, 2 * CW], F32, tag="z", name=f"z_{c}")
            for f in range(2):
                pz = pz_pool.tile([128, CW], F32, tag="pz", name=f"pz{f}_{c}")
                nc.tensor.matmul(pz[:, :], fcols(wt["wiz"][0], f), xt[0][:, :],
                                 start=True, stop=False)
                nc.tensor.matmul(pz[:, :], fcols(wt["wiz"][1], f), xt[1][:, :],
                                 start=False, stop=False)
                nc.tensor.matmul(pz[:, :], fcols(wt["whz"][0], f), ht[:, 0:CW],
                                 start=False, stop=False)
                nc.tensor.matmul(pz[:, :], fcols(wt["whz"][1], f), ht[:, CW:2 * CW],
                                 start=False, stop=True)
                nc.scalar.activation(zt[:, f * CW:(f + 1) * CW], pz[:, :],
                                     mybir.ActivationFunctionType.Sigmoid,
                                     bias=bias_t[:, f * 3 + 1:f * 3 + 2])

            # --- xr = Wir@x, kept in SBUF as float32r for the identity-add ---
            xr = []
            for f in range(2):
                pxr = pr_pool.tile([128, CW], F32, tag="pr", name=f"pxr{f}_{c}")
                nc.tensor.matmul(pxr[:, :], fcols(wt["wir"][0], f), xt[0][:, :],
                                 start=True, stop=False)
                nc.tensor.matmul(pxr[:, :], fcols(wt["wir"][1], f), xt[1][:, :],
                                 start=False, stop=True)
                x_s = xr_pool.tile([128, CW], F32R, tag=f"xr{f}", name=f"xr{f}_{c}")
                nc.scalar.copy(x_s[:, :], pxr[:, :])
                xr.append(x_s)

            # --- neighbor loop: r_cat slices = sigmoid(Whr@hs_n + xr + b_r) ---
            rc = r_pool.tile([128, 2 * N_NEIGH * CW], F32, tag="r", name=f"r_{c}")
            for n in range(N_NEIGH):
                for f in range(2):
                    pr = pr_pool.tile([128, CW], F32, tag="pr", name=f"pr{n}{f}_{c}")
                    nc.tensor.matmul(pr[:, :], fcols(wt["whr"][0], f),
                                     hs_slice(0, n), start=True, stop=False)
                    nc.tensor.matmul(pr[:, :], fcols(wt["whr"][1], f),
                                     hs_slice(1, n), start=False, stop=False)
                    nc.tensor.matmul(pr[:, :], id_t[:, :], xr[f][:, :],
                                     start=False, stop=True)
                    nc.scalar.activation(
                        rc[:, (f * N_NEIGH + n) * CW:(f * N_NEIGH + n + 1) * CW],
                        pr[:, :], mybir.ActivationFunctionType.Sigmoid,
                        bias=bias_t[:, f * 3:f * 3 + 1])

            # --- s = sum_n r_n * hs_n: one big in-place multiply + one
            #     neighbor-axis reduce; s is [128, 1024] with f halves ---
            nc.vector.tensor_mul(rc[:, :], rc[:, :], hsc[:, :].bitcast(F32))
            sc = s_pool.tile([128, 2 * CW], F32R, tag="s", name=f"s_{c}")
            with nc.allow_low_precision(reason="float32r tile; reduce is fp32 internally"):
                nc.vector.tensor_reduce(
                    sc[:, :].rearrange("p (f b) -> p f b", f=2),
                    rc[:, :].rearrange("p (f n b) -> p f b n", f=2, n=N_NEIGH),
                    axis=mybir.AxisListType.X, op=mybir.AluOpType.add)

            # --- n gate: tanh(Win@x + Whn@s + b_n), PSUM-accumulated ---
            nt = n_pool.tile([128, 2 * CW], F32, tag="n", name=f"n_{c}")
            for f in range(2):
                pn = pn_pool.tile([128, CW], F32, tag="pn", name=f"pn{f}_{c}")
                nc.tensor.matmul(pn[:, :], fcols(wt["win"][0], f), xt[0][:, :],
                                 start=True, stop=False)
                nc.tensor.matmul(pn[:, :], fcols(wt["win"][1], f), xt[1][:, :],
                                 start=False, stop=False)
                nc.tensor.matmul(pn[:, :], fcols(wt["whn"][0], f), sc[:, 0:CW],
                                 start=False, stop=False)
                nc.tensor.matmul(pn[:, :], fcols(wt["whn"][1], f), sc[:, CW:2 * CW],
                                 start=False, stop=True)
                nc.scalar.activation(nt[:, f * CW:(f + 1) * CW], pn[:, :],
                                     mybir.ActivationFunctionType.Tanh,
                                     bias=bias_t[:, f * 3 + 2:f * 3 + 3])

            # --- out = n + z * (h - n) on GPSIMD, [128, 1024] ---
            dt_ = d_pool.tile([128, 2 * CW], F32, tag="d", name=f"d_{c}")
            nc.gpsimd.tensor_sub(dt_[:, :], ht[:, :].bitcast(F32), nt[:, :])
            nc.gpsimd.tensor_mul(dt_[:, :], zt[:, :], dt_[:, :])
            ot = o_pool.tile([128, 2 * CW], F32, tag="o", name=f"o_{c}")
            nc.gpsimd.tensor_add(ot[:, :], nt[:, :], dt_[:, :])
            for f in range(2):
                nc.sync.dma_start(out=outT[f * 128:(f + 1) * 128, sl],
                                  in_=ot[:, f * CW:(f + 1) * CW])

    nc.compile()
    return nc


def _prep_inputs(x, h_sum, hs, Wir, bir, Whr, bhr, Wiz, biz, Whz, bhz,
                 Win, bin_, Whn, bhn):
    """Shard + transpose to feature-major per-core input maps."""
    f32 = np.float32
    xT = np.ascontiguousarray(np.asarray(x, f32).T)          # [256, B]
    hT = np.ascontiguousarray(np.asarray(h_sum, f32).T)      # [256, B]
    hsT = np.ascontiguousarray(np.asarray(hs, f32).transpose(0, 2, 1))  # [8,256,B]

    w = {
        "wir": np.ascontiguousarray(np.asarray(Wir, f32).T),
        "whr": np.ascontiguousarray(np.asarray(Whr, f32).T),
        "wiz": np.ascontiguousarray(np.asarray(Wiz, f32).T),
        "whz": np.ascontiguousarray(np.asarray(Whz, f32).T),
        "win": np.ascontiguousarray(np.asarray(Win, f32).T),
        "whn": np.ascontiguousarray(np.asarray(Whn, f32).T),
    }
    b_r = np.asarray(bir, f32) + np.asarray(bhr, f32)
    b_z = np.asarray(biz, f32) + np.asarray(bhz, f32)
    b_n = np.asarray(bin_, f32) + np.asarray(bhn, f32)
    biasp = np.empty((128, 6), f32)
    for f in range(2):
        biasp[:, f * 3 + 0] = b_r[f * 128:(f + 1) * 128]
        biasp[:, f * 3 + 1] = b_z[f * 128:(f + 1) * 128]
        biasp[:, f * 3 + 2] = b_n[f * 128:(f + 1) * 128]
    ident = np.eye(128, dtype=f32)

    in_maps = []
    for c in range(M):
        sl = slice(c * BL, (c + 1) * BL)
        m = {
            "xT": np.ascontiguousarray(xT[:, sl]),
            "hT": np.ascontiguousarray(hT[:, sl]),
            "hsT": np.ascontiguousarray(hsT[:, :, sl]),
            "ident": ident,
            "biasp": biasp,
        }
        m.update(w)
        in_maps.append(m)
    return in_maps


def _run(inputs, trace=False, **trace_kwargs):
    global _cached
    if _cached is None:
        _cached = _build()
    nc = _cached
    in_maps = _prep_inputs(**inputs)
    res = run_bass_kernel_spmd(nc, in_maps, list(range(M)), trace=trace,
                               **trace_kwargs)
    out = np.empty((B, H), np.float32)
    for c in range(M):
        out[c * BL:(c + 1) * BL, :] = res.results[c]["outT"].T
    return out, res


def kernel(**inputs):
    return _run(inputs)[0]
